# revision 1
# baseline (speedup 1.0000x reference)
"""Differential attention + quirky GroupNorm + output proj on 8 TRN2 NeuronCores.

Strategy (head-sharded attention, token-sharded norm+proj), all-bf16 PE:
  - Host preps transposed layouts (xT, per-core wqkvT slice with the
    1/sqrt(hd) attention scale folded into the K rows, woT) plus RoPE
    cos/sin tables, band masks, identity, and wo row-sums.
  - Stage 1: each core computes QKV for its 2 heads in bf16 (fp32 DMA +
    on-device bf16 convert), applies RoPE, stores V (token-major, bf16)
    via DRAM for the per-head attention gather.
  - Stage 2: per (head, batch): scores in half-paired row-tiled bf16
    matmuls into 2-bank PSUM groups, +1.0-above-diagonal mask added via
    an identity-weight matmul on the 4 diagonal band tiles, one exp per
    [128,1024] group, softmax denominators via col-paired ones-matmuls
    sharing a PSUM bank, reciprocal_approx_fast + gpsimd partition
    broadcast for the per-query scales, PV in bf16, differential combine
    via one scalar_tensor_tensor with -lambda.  AllToAll (bf16) per head
    redistributes to token sharding.
  - Stage 4 (mostly after attention): GroupNorm stats (groups are
    128-token blocks x all channels), projection in bf16 with the
    beta/mean terms as a K=1 matmul against host-provided wo row-sums.
"""

import math
from contextlib import ExitStack

import numpy as np

import concourse.bass as bass
import concourse.tile as tile
from concourse import bacc, mybir
from concourse.bass_utils import run_bass_kernel_spmd

F32 = mybir.dt.float32
F32R = mybir.dt.float32r
BF16 = mybir.dt.bfloat16
AX = mybir.AxisListType
OP = mybir.AluOpType
ACTF = mybir.ActivationFunctionType

B, S, E, H = 2, 2048, 2048, 16
HD = E // H                # 128
NC = 8                     # cores
HPC = H // NC              # 2 heads per core
CPC = HPC * HD             # 256 channels per core
T = B * S                  # 4096 tokens
TPC = T // NC              # 512 tokens per core
NG = TPC // HD             # 4 groups per core (128-token blocks)
GROUP_N = float(HD * E)    # 262144 elements per group
INIT_LAMBDA = 0.8
EPS = 1e-5
SCALER = HD ** -0.5


def _emit_sumpv(nc, g, sum_t, pv, ps_o, ones_col_bf, v_bh, pts, last=False):
    """Sums + PV matmuls for ktpair group g (both halves).

    The two halves' sum chains live in separate PSUM banks but at array
    column positions 0 and 32 (out partition 0 vs 32), so consecutive sum
    matmuls run concurrently in distinct column groups of the PE array."""
    for half in range(2):
        pt = pts[half][g]
        row = half * 32
        for j in range(2):
            nc.tensor.matmul(
                sum_t[half][row:row + 1, :],
                ones_col_bf[:],
                pt[:, j * 512:(j + 1) * 512],
                start=(g == 0 and j == 0), stop=(last and j == 1),
            )
    for half in range(2):
        pt = pts[half][g]
        if pv[half] is None:
            pv[half] = ps_o.tile([128, 512], F32, tag=f"pv{half}",
                                 name=f"pv{half}")
        for j in range(2):
            kt = 2 * g + j
            nc.tensor.matmul(
                pv[half][:],
                v_bh(kt),
                pt[:, j * 512:(j + 1) * 512],
                start=(g == 0 and j == 0), stop=(last and j == 1),
            )


def _attn_qc(nc, P, q_sb, k_sb, v_bh, bh, qc, ident_bf, masks_bf,
             ones_col_bf, lamneg_col, a2a_in_h):
    """One 512-query chunk of differential attention for (head, batch)."""
    sum_t0 = P["ps_sum"].tile([64, 512], F32, tag="sum_t0")
    sum_t1 = P["ps_sum"].tile([64, 512], F32, tag="sum_t1")
    sum_t = [sum_t0, sum_t1]
    pv = [None, None]
    pts = [[], []]
    pending = None
    for g in range(8):
        kt0 = 2 * g
        for half in range(2):
            hs = half * 64
            q_ap = q_sb[hs:hs + 64, bh * S + qc * 512:bh * S + (qc + 1) * 512]
            scg = P["ps_sc"].tile([128, 1024], F32, tag="scg")
            for j in range(2):
                kt = kt0 + j
                o = kt - 4 * qc
                diag = 0 <= o < 4
                sl = scg[:, j * 512:(j + 1) * 512]
                nc.tensor.matmul(
                    sl,
                    k_sb[hs:hs + 64,
                         bh * S + kt * 128:bh * S + kt * 128 + 128],
                    q_ap,
                    start=True, stop=not diag,
                )
                if diag:
                    nc.tensor.matmul(
                        sl, ident_bf[:], masks_bf[:, o * 512:(o + 1) * 512],
                        start=False, stop=True,
                    )
            ptp = P["pt0"] if half == 0 else P["pt1"]
            pt = ptp.tile([128, 1024], BF16, tag=f"pt{half}")
            bias = 1.0 if (kt0 - 4 * qc) >= 4 else 0.0
            nc.scalar.activation(pt[:], scg[:], ACTF.Exp, bias=bias)
            pts[half].append(pt)
        # previous group's sums+PV emitted after this group's scores so the
        # tensor engine always has ready work while ACT runs the exps
        if pending is not None:
            _emit_sumpv(nc, pending, sum_t, pv, P["ps_o"], ones_col_bf,
                        v_bh, pts)
        pending = g
    _emit_sumpv(nc, pending, sum_t, pv, P["ps_o"], ones_col_bf, v_bh, pts,
                last=True)

    # normalize + combine.  reciprocal_approx_fast (and partition_broadcast)
    # are only correct at partition base 0 on HW, so the half1 sum row is
    # first moved from partition 32 to 0 via ScalarE (partition-shift-proven).
    recip0 = P["recipp"].tile([1, 512], F32, tag="recip0")
    nc.vector.reciprocal_approx_fast(recip0[:], sum_t0[0:1, :])
    sum1_sb = P["recipp"].tile([1, 512], F32, tag="sum1_sb")
    nc.scalar.copy(sum1_sb[:], sum_t1[32:33, :])
    recip1 = P["recipp"].tile([1, 512], F32, tag="recip1")
    nc.vector.reciprocal_approx_fast(recip1[:], sum1_sb[:])
    bcs0 = P["bcsp"].tile([128, 512], F32, tag="bcs")
    nc.gpsimd.partition_broadcast(bcs0[:], recip0[0:1, :])
    bcs1 = P["bcsp"].tile([128, 512], F32, tag="bcs")
    nc.gpsimd.partition_broadcast(bcs1[:], recip1[0:1, :])
    th0 = P["thp"].tile([128, 512], F32, tag="th0")
    nc.vector.tensor_tensor(th0[:], pv[0][:], bcs0[:], OP.mult)
    th1 = P["thp"].tile([128, 512], F32, tag="th1")
    nc.vector.tensor_tensor(th1[:], pv[1][:], bcs1[:], OP.mult)
    a_sb = P["asbp"].tile([128, 512], BF16, tag="a_sb")
    nc.vector.scalar_tensor_tensor(
        out=a_sb[:], in0=th1[:], scalar=lamneg_col[:, 0:1],
        in1=th0[:], op0=OP.mult, op1=OP.add,
    )
    nc.sync.dma_start(a2a_in_h[2 * qc, :, :], a_sb[:, 0:256])
    nc.sync.dma_start(a2a_in_h[2 * qc + 1, :, :], a_sb[:, 256:512])


def _stage1(nc, P, q_sb, k_sb, w_bf, cos_sb, sin_sb, xT, wqkvT, cosd, sind):
    for tci in range(T // 512):
        b = tci // 4
        sc = tci % 4
        xbfs = []
        for et in range(16):
            if tci == 0:
                # interleave weight chunks with the first x chunks so the
                # first matmul's inputs arrive after ~0.8MB of DMA, not 8MB
                ws = P["wstage"].tile([128, 3 * CPC], F32, tag="ws")
                nc.sync.dma_start(
                    ws[:].bitcast(F32R),
                    wqkvT[et * 128:(et + 1) * 128, :].bitcast(F32R),
                )
                nc.vector.tensor_copy(
                    w_bf[:, et * 3 * CPC:(et + 1) * 3 * CPC], ws[:]
                )
            xs = P["xstage"].tile([128, 512], F32, tag="xs")
            nc.sync.dma_start(
                xs[:].bitcast(F32R),
                xT[et * 128:(et + 1) * 128,
                   tci * 512:(tci + 1) * 512].bitcast(F32R),
            )
            xb = P["xbf"].tile([128, 512], BF16, tag="xb")
            if et % 2 == 0:
                nc.vector.tensor_copy(xb[:], xs[:])
            else:
                nc.scalar.copy(xb[:], xs[:])
            xbfs.append(xb)
        if tci == 0:
            nc.sync.dma_start(cos_sb[:], cosd[:, :])
            nc.sync.dma_start(sin_sb[:], sind[:, :])
        elif tci == 1 and P.get("post_tci0"):
            P["post_tci0"]()
        # q, k channel-major [hd, 512 tokens] per local head
        for which, dst in ((0, q_sb), (1, k_sb)):
            for h in range(HPC):
                ps = P["ps_qk"].tile([128, 512], F32, tag="psqk")
                for et in range(16):
                    wcol = et * 3 * CPC + which * CPC + h * HD
                    nc.tensor.matmul(
                        ps[:], w_bf[:, wcol:wcol + HD], xbfs[et][:],
                        start=(et == 0), stop=(et == 15),
                    )
                csl = cos_sb[:, sc * 512:(sc + 1) * 512]
                ssl = sin_sb[:, sc * 512:(sc + 1) * 512]
                qc_t = P["s1tmp"].tile([128, 512], F32, tag="ropeqc")
                nc.vector.tensor_tensor(qc_t[:], ps[:], csl, OP.mult)
                rot = P["s1tmp"].tile([128, 512], F32, tag="roperot")
                nc.scalar.copy(rot[0:64, :], ps[64:128, :])
                nc.scalar.copy(rot[64:128, :], ps[0:64, :])
                nc.vector.tensor_tensor(rot[:], rot[:], ssl, OP.mult)
                col = (b * HPC + h) * S + sc * 512
                nc.vector.tensor_tensor(dst[:, col:col + 512], qc_t[:],
                                        rot[:], OP.add)
        # v token-major [t, 256], kept resident in SBUF (no DRAM bounce)
        v_sb = P["v_sb"]
        for ts4 in range(4):
            ps = P["ps_v"].tile([128, CPC], F32, tag="psv")
            for et in range(16):
                wcol = et * 3 * CPC + 2 * CPC
                nc.tensor.matmul(
                    ps[:], xbfs[et][:, ts4 * 128:(ts4 + 1) * 128],
                    w_bf[:, wcol:wcol + CPC],
                    start=(et == 0), stop=(et == 15),
                )
            blk = tci * 4 + ts4
            nc.scalar.copy(v_sb[:, blk * CPC:(blk + 1) * CPC], ps[:])


def _stage4_finalize(nc, P, ones_col, ones_row, g_sb, st_sum, st_sq,
                     gb, rowsum, woT, out):
    s4res = P["s4res"]
    red8 = s4res.tile([128, 2 * NG], F32, tag="red8")
    nc.vector.tensor_reduce(red8[:, 0:NG].bitcast(F32R), st_sum[:], AX.X,
                            OP.add)
    nc.vector.tensor_reduce(
        red8[:, NG:2 * NG].bitcast(F32R),
        st_sq[:].rearrange("p (g c) -> p g c", g=NG), AX.X, OP.add
    )
    stat_ps = P["ps_st"].tile([1, 2 * NG], F32, tag="statp")
    nc.tensor.matmul(stat_ps[:], ones_col[:].bitcast(F32R),
                     red8[:].bitcast(F32R), start=True, stop=True)
    srow = s4res.tile([1, 2 * NG], F32, tag="srow")
    nc.vector.tensor_scalar(out=srow[:], in0=stat_ps[:],
                            scalar1=1.0 / GROUP_N, scalar2=None, op0=OP.mult)
    var_r = s4res.tile([1, NG], F32, tag="var_r")
    m2 = s4res.tile([1, NG], F32, tag="m2")
    nc.vector.tensor_tensor(m2[:], srow[:, 0:NG], srow[:, 0:NG], OP.mult)
    nc.vector.tensor_tensor(var_r[:], srow[:, NG:2 * NG], m2[:], OP.subtract)
    eps_t = s4res.tile([1, 1], F32, tag="eps_t")
    nc.gpsimd.memset(eps_t[:], EPS)
    std_r = s4res.tile([1, NG], F32, tag="std_r")
    nc.scalar.activation(std_r[:], var_r[:], ACTF.Sqrt, bias=eps_t[:])
    ab_row = s4res.tile([1, 2 * NG], F32, tag="ab_row")
    nc.vector.reciprocal(ab_row[:, 0:NG].bitcast(F32R), std_r[:])
    mtmp = s4res.tile([1, NG], F32, tag="mtmp")
    nc.vector.tensor_tensor(mtmp[:], srow[:, 0:NG], ab_row[:, 0:NG], OP.mult)
    nc.vector.tensor_scalar(
        out=ab_row[:, NG:2 * NG].bitcast(F32R), in0=mtmp[:],
        scalar1=-1.0, scalar2=None, op0=OP.mult,
    )  # b_g = -mean*rstd
    ab_ps = P["ps_st"].tile([128, 2 * NG], F32, tag="abp")
    nc.tensor.matmul(ab_ps[:], ones_row[:].bitcast(F32R),
                     ab_row[:].bitcast(F32R), start=True, stop=True)
    ab_bc = s4res.tile([128, 2 * NG], F32, tag="ab_bc")
    nc.scalar.copy(ab_bc[:], ab_ps[:])

    gamma_col = s4res.tile([128, NG], F32, tag="gamma_col")
    nc.sync.dma_start(gamma_col[:],
                      gb[0, :].rearrange("(c p) -> p c", p=128))
    m1_col = s4res.tile([128, NG], F32, tag="m1_col")
    nc.vector.scalar_tensor_tensor(
        out=m1_col[:], in0=gamma_col[:], scalar=(1.0 - INIT_LAMBDA),
        in1=ab_bc[:, 0:NG], op0=OP.mult, op1=OP.mult,
    )
    gamma_row = s4res.tile([1, TPC], F32, tag="gamma_row")
    nc.sync.dma_start(gamma_row[:], gb[0:1, :])
    beta_row = s4res.tile([1, TPC], F32, tag="beta_row")
    nc.sync.dma_start(beta_row[:], gb[1:2, :])
    m2_row = s4res.tile([1, TPC], F32, tag="m2_row")
    m2tmp = s4res.tile([1, TPC], F32, tag="m2tmp")
    nc.vector.tensor_tensor(
        m2tmp[:].rearrange("o (g u) -> o g u", g=NG),
        gamma_row[:].rearrange("o (g u) -> o g u", g=NG),
        ab_row[:, NG:2 * NG].rearrange("o (g u) -> o g u", u=1)
        .to_broadcast([1, NG, HD]),
        OP.mult,
    )
    nc.vector.tensor_tensor(m2tmp[:], m2tmp[:], beta_row[:], OP.add)
    nc.vector.tensor_scalar(out=m2_row[:], in0=m2tmp[:],
                            scalar1=(1.0 - INIT_LAMBDA), scalar2=None,
                            op0=OP.mult)
    m2_bf = s4res.tile([1, TPC], BF16, tag="m2_bf")
    nc.vector.tensor_copy(m2_bf[:], m2_row[:])
    rs_f = s4res.tile([1, E], F32, tag="rs_f")
    nc.sync.dma_start(rs_f[:], rowsum[:, :])
    rs_bf = s4res.tile([1, E], BF16, tag="rs_bf")
    nc.vector.tensor_copy(rs_bf[:], rs_f[:])

    for oc in range(4):
        if oc == 0 and "wots0" in P:
            wots = P["wots0"]
        else:
            wots = []
            for ct in range(16):
                wstg = P["wostage"].tile([128, 512], F32, tag="wstg")
                nc.sync.dma_start(
                    wstg[:].bitcast(F32R),
                    woT[ct * 128:(ct + 1) * 128,
                        oc * 512:(oc + 1) * 512].bitcast(F32R),
                )
                wot = P["wop"].tile([128, 512], BF16, tag="wot")
                nc.vector.tensor_copy(wot[:], wstg[:])
                wots.append(wot)
        for tcg in range(NG):
            # group tcg = (batch tcg//2, block tcg%2); g_sb is b-major
            po = P["ps_p"].tile([128, 512], F32, tag="po")
            for ct in range(16):
                col = ((tcg // 2) * 16 + ct) * (TPC // B) + (tcg % 2) * 128
                nc.tensor.matmul(
                    po[:],
                    g_sb[:, col:col + 128],
                    wots[ct][:],
                    start=(ct == 0), stop=(ct == 15),
                )
            bps = P["ps_b"].tile([128, 512], F32, tag="bps")
            nc.tensor.matmul(
                bps[:], m2_bf[0:1, tcg * 128:(tcg + 1) * 128],
                rs_bf[0:1, oc * 512:(oc + 1) * 512], start=True, stop=True,
            )
            osb = P["s4tmp"].tile([128, 512], F32, tag="osb")
            nc.scalar.activation(osb[:], po[:], ACTF.Copy,
                                 scale=m1_col[:, tcg:tcg + 1])
            nc.vector.tensor_tensor(osb[:], osb[:], bps[:], OP.add)
            nc.sync.dma_start(
                out[tcg * 128:(tcg + 1) * 128, oc * 512:(oc + 1) * 512],
                osb[:],
            )


def build_nc():
    nc = bacc.Bacc("TRN2", target_bir_lowering=False, debug=False,
                   num_devices=NC)

    xT = nc.declare_dram_parameter("xT", [E, T], F32, isOutput=False)
    wqkvT = nc.declare_dram_parameter("wqkvT", [E, 3 * CPC], F32,
                                      isOutput=False)
    woT = nc.declare_dram_parameter("woT", [E, E], F32, isOutput=False)
    cosd = nc.declare_dram_parameter("cosd", [HD, S], F32, isOutput=False)
    sind = nc.declare_dram_parameter("sind", [HD, S], F32, isOutput=False)
    bandm = nc.declare_dram_parameter("bandm", [4, 128, 512], F32,
                                      isOutput=False)
    identm = nc.declare_dram_parameter("identm", [128, 128], F32,
                                       isOutput=False)
    lam_a = nc.declare_dram_parameter("lam_a", [2, HD], F32, isOutput=False)
    lam_b = nc.declare_dram_parameter("lam_b", [2, HD], F32, isOutput=False)
    sgn2 = nc.declare_dram_parameter("sgn2", [2, 1], F32, isOutput=False)
    ones128 = nc.declare_dram_parameter("ones128", [1, 128], F32,
                                        isOutput=False)
    rowsum = nc.declare_dram_parameter("rowsum", [1, E], F32, isOutput=False)
    gb = nc.declare_dram_parameter("gb", [2, TPC], F32, isOutput=False)
    out = nc.declare_dram_parameter("out", [TPC, E], F32, isOutput=True)

    TPB = TPC // B  # 256 tokens per (core, batch)
    a2a_in = [[nc.dram_tensor(f"a2a_in{h}_{b}", [NC, HD, TPB], BF16)
               for b in range(B)] for h in range(HPC)]
    a2a_out = [[nc.dram_tensor(f"a2a_out{h}_{b}", [NC, HD, TPB], BF16)
                for b in range(B)] for h in range(HPC)]

    with tile.TileContext(nc) as tc, \
         nc.allow_low_precision(reason="bf16 matmul paths; fp32 accumulate"), \
         ExitStack() as top:
        small = top.enter_context(tc.tile_pool(name="small", bufs=1))
        ident_bf = small.tile([128, 128], BF16, tag="ident_bf")
        masks_bf = small.tile([128, 4 * 512], BF16, tag="masks_bf")

        qkres = top.enter_context(tc.tile_pool(name="qkres", bufs=1))
        q_sb = qkres.tile([128, 2 * HPC * S], BF16, tag="q_sb")
        k_sb = qkres.tile([128, 2 * HPC * S], BF16, tag="k_sb")
        # V kept resident in SBUF: 32 token tiles x 256 channels, bf16
        v_sb = qkres.tile([128, 32 * CPC], BF16, tag="v_sb")

        # ======== stage 1: QKV + RoPE ========
        with ExitStack() as s1:
            P1 = {}
            P1["s1res"] = s1.enter_context(tc.tile_pool(name="s1res", bufs=1))
            P1["xstage"] = s1.enter_context(tc.tile_pool(name="xstage",
                                                         bufs=5))
            P1["xbf"] = s1.enter_context(tc.tile_pool(name="xbf", bufs=20))
            P1["s1tmp"] = s1.enter_context(tc.tile_pool(name="s1tmp", bufs=6))
            P1["ps_qk"] = s1.enter_context(
                tc.tile_pool(name="ps_qk", bufs=5, space="PSUM"))
            P1["ps_v"] = s1.enter_context(
                tc.tile_pool(name="ps_v", bufs=3, space="PSUM"))

            w_bf = P1["s1res"].tile([128, 16 * 3 * CPC], BF16, tag="w_bf")
            cos_sb = P1["s1res"].tile([HD, S], F32, tag="cos_sb")
            sin_sb = P1["s1res"].tile([HD, S], F32, tag="sin_sb")
            P1["wstage"] = s1.enter_context(tc.tile_pool(name="wstage",
                                                         bufs=3))

            def post_tci0():
                # stage-2 constants staged after the first token chunk so
                # their DMAs don't delay the stage-1 critical path
                idf = P1["wstage"].tile([128, 128], F32, tag="idf",
                                        name="idf")
                nc.sync.dma_start(idf[:].bitcast(F32R),
                                  identm[:, :].bitcast(F32R))
                nc.vector.tensor_copy(ident_bf[:], idf[:])
                mf = P1["wstage"].tile([128, 4 * 512], F32, tag="mf",
                                       name="mf")
                for o in range(4):
                    nc.sync.dma_start(mf[:, o * 512:(o + 1) * 512],
                                      bandm[o, :, :])
                nc.vector.tensor_copy(masks_bf[:], mf[:])

            P1["post_tci0"] = post_tci0
            P1["v_sb"] = v_sb
            _stage1(nc, P1, q_sb, k_sb, w_bf, cos_sb, sin_sb, xT, wqkvT,
                    cosd, sind)

        # ---- constants needed only from stage 2 on: emitted after stage 1
        # so their DMAs and the lambda matmul don't head-of-line block the
        # stage-1 DMA and PE queues ----
        ones_col = small.tile([128, 1], F32, tag="ones_col")
        nc.sync.dma_start(
            ones_col[:].bitcast(F32R),
            ones128[0, :].rearrange("(p o) -> p o", o=1).bitcast(F32R),
        )
        ones_row = small.tile([1, 128], F32, tag="ones_row")
        nc.sync.dma_start(ones_row[:].bitcast(F32R),
                          ones128[:, :].bitcast(F32R))
        ones_col_bf = small.tile([128, 1], BF16, tag="ones_col_bf")
        nc.vector.tensor_copy(ones_col_bf[:], ones_col[:])

        # ---- lambda scalar ----
        la = small.tile([2, HD], F32, tag="la")
        nc.sync.dma_start(la[:], lam_a[:, :])
        lb = small.tile([2, HD], F32, tag="lb")
        nc.sync.dma_start(lb[:], lam_b[:, :])
        prod = small.tile([2, HD], F32, tag="lprod")
        nc.vector.tensor_tensor(prod[:], la[:], lb[:], OP.mult)
        dots = small.tile([2, 1], F32, tag="ldots")
        nc.vector.tensor_reduce(
            dots[:], prod[:].rearrange("p (n u) -> p n u", u=HD), AX.X, OP.add
        )
        lexp = small.tile([2, 1], F32, tag="lexp")
        nc.scalar.activation(lexp[:], dots[:], ACTF.Exp)
        sv = small.tile([2, 1], F32, tag="sv")
        nc.sync.dma_start(sv[:], sgn2[:, :])
        with tc.tile_pool(name="ps_lam", bufs=1, space="PSUM") as ps_lam:
            lam_ps = ps_lam.tile([1, 1], F32, tag="lam_ps")
            nc.tensor.matmul(lam_ps[:], sv[:], lexp[:], start=True, stop=True)
            lam_t = small.tile([1, 1], F32, tag="lam_t")
            nc.vector.tensor_scalar(out=lam_t[:], in0=lam_ps[:],
                                    scalar1=INIT_LAMBDA, scalar2=None,
                                    op0=OP.add)
        lam_col = small.tile([128, 1], F32, tag="lam_col")
        nc.gpsimd.partition_broadcast(lam_col[:], lam_t[0:1, :])
        lamneg_col = small.tile([128, 1], F32, tag="lamneg_col")
        nc.vector.tensor_scalar(out=lamneg_col[:], in0=lam_col[:],
                                scalar1=-1.0, scalar2=None, op0=OP.mult)

        # ======== stage 2 (+ stage 4 SBUF) ========
        with ExitStack() as s2:
            P = {}
            for name, bufs in (("pt0", 5), ("pt1", 5),
                               ("recipp", 3), ("bcsp", 4), ("thp", 4),
                               ("asbp", 3), ("s4res", 1), ("sqscratch", 2),
                               ("wostage", 4), ("wop", 20), ("s4tmp", 4)):
                P[name] = s2.enter_context(tc.tile_pool(name=name, bufs=bufs))
            with ExitStack() as s2p:
                P["ps_sc"] = s2p.enter_context(
                    tc.tile_pool(name="ps_sc", bufs=2, space="PSUM"))
                P["ps_sum"] = s2p.enter_context(
                    tc.tile_pool(name="ps_sum", bufs=1, space="PSUM"))
                P["ps_o"] = s2p.enter_context(
                    tc.tile_pool(name="ps_o", bufs=1, space="PSUM"))

                TPB = TPC // B  # 256
                NGB = TPB // HD  # 2 groups per (core, batch)
                g_sb = P["s4res"].tile([128, B * 16 * TPB], BF16, tag="g_sb")
                st_sum = P["s4res"].tile([128, B * NGB, 16], F32, tag="st_sum")
                st_sq = P["s4res"].tile([128, B * NGB * 16], F32, tag="st_sq")
                def emit_stats(b, parity=None):
                    # G loads + stat partials for batch b (no PSUM).
                    # parity selects channels of one local head (ct%2) whose
                    # A2A has already fired, so they can be emitted early.
                    for ct in range(16):
                        if parity is not None and ct % 2 != parity:
                            continue
                        col = (b * 16 + ct) * TPB
                        nc.sync.dma_start(
                            g_sb[:, col:col + TPB],
                            a2a_out[ct % 2][b][ct // 2, :, :],
                        )
                        gt = g_sb[:, col:col + TPB]
                        nc.vector.tensor_reduce(
                            st_sum[:, b * NGB:(b + 1) * NGB, ct],
                            gt.rearrange("p (g n) -> p g n", g=NGB),
                            AX.X, OP.add,
                        )
                        for g in range(NGB):
                            scr = P["sqscratch"].tile([128, HD], F32,
                                                      tag="sqs", name="sqs")
                            gi = b * NGB + g
                            nc.scalar.activation(
                                scr[:], gt[:, g * HD:(g + 1) * HD],
                                ACTF.Square,
                                accum_out=st_sq[:, gi * 16 + ct:
                                                gi * 16 + ct + 1],
                            )

                for b, h in ((0, 0), (0, 1), (1, 0), (1, 1)):
                    bh = b * HPC + h

                    def v_bh(kt, b=b, h=h):
                        base = (b * 16 + kt) * CPC + h * HD
                        return v_sb[:, base:base + HD]

                    for qc in range(4):
                        _attn_qc(nc, P, q_sb, k_sb, v_bh, bh, qc,
                                 ident_bf, masks_bf, ones_col_bf,
                                 lamneg_col, a2a_in[h][b])
                        if (b, h) == (1, 1) and qc == 0:
                            # b1's h0 channels arrived with A2A(h0,b1);
                            # emit their stats here so only the h1 half
                            # remains after the final collective
                            emit_stats(1, parity=0)
                    nc.gpsimd.collective_compute(
                        "AllToAll",
                        OP.bypass,
                        replica_groups=[list(range(NC))],
                        ins=[a2a_in[h][b].ap().opt()],
                        outs=[a2a_out[h][b].ap().opt()],
                    )
                    if (b, h) == (1, 0):
                        # b0 stats emitted here: their ACT/DVE ops queue
                        # behind (1,0)'s exps, with the b0 A2As long done —
                        # no head-of-line block on the ACT queue
                        emit_stats(0)
                        # prefetch the first projection weight block so the
                        # tail doesn't start with 2MB of wo DMA + converts
                        wots0 = []
                        for ct in range(16):
                            wstg = P["wostage"].tile([128, 512], F32,
                                                     tag="wstg", name="wstg")
                            nc.sync.dma_start(
                                wstg[:].bitcast(F32R),
                                woT[ct * 128:(ct + 1) * 128,
                                    0:512].bitcast(F32R),
                            )
                            wot = P["wop"].tile([128, 512], BF16,
                                                tag="wot", name="wot")
                            nc.vector.tensor_copy(wot[:], wstg[:])
                            wots0.append(wot)
                        P["wots0"] = wots0
                emit_stats(1, parity=1)

            # ======== stage 4: stats finalize + projection ========
            with ExitStack() as s4p:
                P["ps_st"] = s4p.enter_context(
                    tc.tile_pool(name="ps_st", bufs=1, space="PSUM"))
                P["ps_b"] = s4p.enter_context(
                    tc.tile_pool(name="ps_b", bufs=2, space="PSUM"))
                P["ps_p"] = s4p.enter_context(
                    tc.tile_pool(name="ps_p", bufs=4, space="PSUM"))
                _stage4_finalize(nc, P, ones_col, ones_row, g_sb, st_sum,
                                 st_sq, gb, rowsum, woT, out)

    nc.compile()
    return nc


_NC_CACHE = None


def _get_nc():
    global _NC_CACHE
    if _NC_CACHE is None:
        _NC_CACHE = build_nc()
    return _NC_CACHE


def _host_prep(x, w_qkv, wo, lambda_q1, lambda_q2, lambda_k1, lambda_k2,
               gamma, beta):
    x = np.asarray(x, dtype=np.float32)
    w_qkv = np.asarray(w_qkv, dtype=np.float32)
    wo = np.asarray(wo, dtype=np.float32)
    gamma = np.asarray(gamma, dtype=np.float32)
    beta = np.asarray(beta, dtype=np.float32)

    xT = np.ascontiguousarray(x.reshape(T, E).T)
    woT = np.ascontiguousarray(wo.T)
    rowsum = np.ascontiguousarray(wo.sum(axis=1)[None, :].astype(np.float32))

    # RoPE tables, channel-major with sign folded into sin
    inv = 1.0 / (10000.0 ** (np.arange(0, HD, 2, dtype=np.float32) / HD))
    ang = np.arange(S, dtype=np.float32)[:, None] * inv[None, :]  # (S, 64)
    ang = np.concatenate([ang, ang], axis=-1)                     # (S, 128)
    cosd = np.ascontiguousarray(np.cos(ang).T.astype(np.float32))  # (128, S)
    sin_t = np.sin(ang).T.astype(np.float32)
    sind = np.ascontiguousarray(
        np.concatenate([-sin_t[:64], sin_t[64:]], axis=0)
    )

    # band masks: mask_o[ki, qi] = 1.0 iff (o*128 + ki) > qi
    o_idx = np.arange(4)[:, None, None] * 128
    ki = np.arange(128)[None, :, None]
    qi = np.arange(512)[None, None, :]
    bandm = ((o_idx + ki) > qi).astype(np.float32)
    identm = np.eye(128, dtype=np.float32)

    lam_a = np.ascontiguousarray(
        np.stack([lambda_q1, lambda_q2]).astype(np.float32)
    )
    lam_b = np.ascontiguousarray(
        np.stack([lambda_k1, lambda_k2]).astype(np.float32)
    )

    in_maps = []
    for j in range(NC):
        h0 = HPC * j
        rows_q = w_qkv[h0 * HD:(h0 + HPC) * HD, :]
        # fold the 1/sqrt(hd) attention scale into the K projection rows
        rows_k = w_qkv[E + h0 * HD:E + (h0 + HPC) * HD, :] * SCALER
        rows_v = w_qkv[2 * E + h0 * HD:2 * E + (h0 + HPC) * HD, :]
        wqkvT_j = np.ascontiguousarray(
            np.concatenate([rows_q.T, rows_k.T, rows_v.T], axis=1)
            .astype(np.float32)
        )
        # core j owns tokens [j*256, (j+1)*256) of each batch; gamma/beta
        # slices duplicated so group gi=(b,g) indexing stays NG=4 on device
        sl = j * (TPC // B)
        gsl = gamma[sl:sl + TPC // B]
        bsl = beta[sl:sl + TPC // B]
        gbj = np.ascontiguousarray(
            np.stack([np.concatenate([gsl, gsl]), np.concatenate([bsl, bsl])])
        )
        in_maps.append({
            "sgn2": np.array([[1.0], [-1.0]], dtype=np.float32),
            "ones128": np.ones((1, 128), dtype=np.float32),
            "xT": xT,
            "wqkvT": wqkvT_j,
            "woT": woT,
            "rowsum": rowsum,
            "cosd": cosd,
            "sind": sind,
            "bandm": bandm,
            "identm": identm,
            "lam_a": lam_a,
            "lam_b": lam_b,
            "gb": gbj,
        })
    return in_maps


def kernel(x, w_qkv, wo, lambda_q1, lambda_q2, lambda_k1, lambda_k2,
           gamma, beta, _trace=False):
    nc = _get_nc()
    in_maps = _host_prep(x, w_qkv, wo, lambda_q1, lambda_q2, lambda_k1,
                         lambda_k2, gamma, beta)
    res = run_bass_kernel_spmd(nc, in_maps, list(range(NC)), trace=_trace)
    out = np.empty((B, S, E), dtype=np.float32)
    tpb = TPC // B
    for j in range(NC):
        rows = res.results[j]["out"].reshape(B, tpb, E)
        for b in range(B):
            out[b, j * tpb:(j + 1) * tpb, :] = rows[b]
    if _trace:
        kernel.last_results = res
    return out



# revision 39
# speedup vs baseline: 1.0363x; 1.0363x over previous
"""Differential attention + quirky GroupNorm + output proj on 8 TRN2 NeuronCores.

Strategy (head-sharded attention, token-sharded norm+proj), all-bf16 PE:
  - Host preps transposed layouts (xT, per-core wqkvT slice with the
    1/sqrt(hd) attention scale folded into the K rows, woT) plus RoPE
    cos/sin tables, band masks, identity, and wo row-sums.
  - Stage 1: each core computes QKV for its 2 heads in bf16 (fp32 DMA +
    on-device bf16 convert), applies RoPE, stores V (token-major, bf16)
    via DRAM for the per-head attention gather.
  - Stage 2: per (head, batch): scores in half-paired row-tiled bf16
    matmuls into 2-bank PSUM groups, +1.0-above-diagonal mask added via
    an identity-weight matmul on the 4 diagonal band tiles, one exp per
    [128,1024] group, softmax denominators via col-paired ones-matmuls
    sharing a PSUM bank, reciprocal_approx_fast + gpsimd partition
    broadcast for the per-query scales, PV in bf16, differential combine
    via one scalar_tensor_tensor with -lambda.  AllToAll (bf16) per head
    redistributes to token sharding.
  - Stage 4 (mostly after attention): GroupNorm stats (groups are
    128-token blocks x all channels), projection in bf16 with the
    beta/mean terms as a K=1 matmul against host-provided wo row-sums.
"""

import math
from contextlib import ExitStack

import numpy as np

import concourse.bass as bass
import concourse.tile as tile
from concourse import bacc, mybir
from concourse.bass_utils import run_bass_kernel_spmd

F32 = mybir.dt.float32
F32R = mybir.dt.float32r
BF16 = mybir.dt.bfloat16
AX = mybir.AxisListType
OP = mybir.AluOpType
ACTF = mybir.ActivationFunctionType

B, S, E, H = 2, 2048, 2048, 16
HD = E // H                # 128
NC = 8                     # cores
HPC = H // NC              # 2 heads per core
CPC = HPC * HD             # 256 channels per core
T = B * S                  # 4096 tokens
TPC = T // NC              # 512 tokens per core
NG = TPC // HD             # 4 groups per core (128-token blocks)
GROUP_N = float(HD * E)    # 262144 elements per group
INIT_LAMBDA = 0.8
EPS = 1e-5
SCALER = HD ** -0.5


def _emit_sumpv(nc, g, sum_t, pv, ps_o, ones_col_bf, v_bh, pts, last=False):
    """Sums + PV matmuls for ktpair group g (both halves).

    The two halves' sum chains live in separate PSUM banks but at array
    column positions 0 and 32 (out partition 0 vs 32), so consecutive sum
    matmuls run concurrently in distinct column groups of the PE array."""
    for half in range(2):
        pt = pts[half][g]
        row = half * 32
        for j in range(2):
            nc.tensor.matmul(
                sum_t[half][row:row + 1, :],
                ones_col_bf[:],
                pt[:, j * 512:(j + 1) * 512],
                start=(g == 0 and j == 0), stop=(last and j == 1),
            )
    for half in range(2):
        pt = pts[half][g]
        if pv[half] is None:
            pv[half] = ps_o.tile([128, 512], F32, tag=f"pv{half}",
                                 name=f"pv{half}")
        for j in range(2):
            kt = 2 * g + j
            nc.tensor.matmul(
                pv[half][:],
                v_bh(kt),
                pt[:, j * 512:(j + 1) * 512],
                start=(g == 0 and j == 0), stop=(last and j == 1),
            )


def _attn_qc(nc, P, q_sb, k_sb, v_bh, bh, qc, ident_bf, masks_bf,
             ones_col_bf, lamneg_col, a2a_in_h):
    """One 512-query chunk of differential attention for (head, batch)."""
    sum_t0 = P["ps_sum"].tile([64, 512], F32, tag="sum_t0")
    sum_t1 = P["ps_sum"].tile([64, 512], F32, tag="sum_t1")
    sum_t = [sum_t0, sum_t1]
    pv = [None, None]
    pts = [[], []]
    pending = None
    for g in range(8):
        kt0 = 2 * g
        for half in range(2):
            hs = half * 64
            q_ap = q_sb[hs:hs + 64, bh * S + qc * 512:bh * S + (qc + 1) * 512]
            scg = P["ps_sc"].tile([128, 1024], F32, tag="scg")
            for j in range(2):
                kt = kt0 + j
                o = kt - 4 * qc
                diag = 0 <= o < 4
                sl = scg[:, j * 512:(j + 1) * 512]
                nc.tensor.matmul(
                    sl,
                    k_sb[hs:hs + 64,
                         bh * S + kt * 128:bh * S + kt * 128 + 128],
                    q_ap,
                    start=True, stop=not diag,
                )
                if diag:
                    nc.tensor.matmul(
                        sl, ident_bf[:], masks_bf[:, o * 512:(o + 1) * 512],
                        start=False, stop=True,
                    )
            ptp = P["pt0"] if half == 0 else P["pt1"]
            pt = ptp.tile([128, 1024], BF16, tag=f"pt{half}")
            bias = 1.0 if (kt0 - 4 * qc) >= 4 else 0.0
            nc.scalar.activation(pt[:], scg[:], ACTF.Exp, bias=bias)
            pts[half].append(pt)
        # previous group's sums+PV emitted after this group's scores so the
        # tensor engine always has ready work while ACT runs the exps
        if pending is not None:
            _emit_sumpv(nc, pending, sum_t, pv, P["ps_o"], ones_col_bf,
                        v_bh, pts)
        pending = g
    _emit_sumpv(nc, pending, sum_t, pv, P["ps_o"], ones_col_bf, v_bh, pts,
                last=True)

    # normalize + combine.  reciprocal_approx_fast (and partition_broadcast)
    # are only correct at partition base 0 on HW, so the half1 sum row is
    # first moved from partition 32 to 0 via ScalarE (partition-shift-proven).
    recip0 = P["recipp"].tile([1, 512], F32, tag="recip0")
    nc.vector.reciprocal_approx_fast(recip0[:], sum_t0[0:1, :])
    sum1_sb = P["recipp"].tile([1, 512], F32, tag="sum1_sb")
    nc.scalar.copy(sum1_sb[:], sum_t1[32:33, :])
    recip1 = P["recipp"].tile([1, 512], F32, tag="recip1")
    nc.vector.reciprocal_approx_fast(recip1[:], sum1_sb[:])
    bcs0 = P["bcsp"].tile([128, 512], F32, tag="bcs")
    nc.gpsimd.partition_broadcast(bcs0[:], recip0[0:1, :])
    bcs1 = P["bcsp"].tile([128, 512], F32, tag="bcs")
    nc.gpsimd.partition_broadcast(bcs1[:], recip1[0:1, :])
    th0 = P["thp"].tile([128, 512], F32, tag="th0")
    nc.vector.tensor_tensor(th0[:], pv[0][:], bcs0[:], OP.mult)
    th1 = P["thp"].tile([128, 512], F32, tag="th1")
    nc.vector.tensor_tensor(th1[:], pv[1][:], bcs1[:], OP.mult)
    a_sb = P["asbp"].tile([128, 512], BF16, tag="a_sb")
    nc.vector.scalar_tensor_tensor(
        out=a_sb[:], in0=th1[:], scalar=lamneg_col[:, 0:1],
        in1=th0[:], op0=OP.mult, op1=OP.add,
    )
    nc.sync.dma_start(a2a_in_h[2 * qc, :, :], a_sb[:, 0:256])
    nc.sync.dma_start(a2a_in_h[2 * qc + 1, :, :], a_sb[:, 256:512])


def _stage1(nc, P, q_sb, k_sb, w_bf, cos_sb, sin_sb, xT, wqkvT, cosd, sind):
    for tci in range(T // 512):
        b = tci // 4
        sc = tci % 4
        xbfs = []
        for et in range(16):
            if tci == 0:
                # interleave weight chunks with the first x chunks so the
                # first matmul's inputs arrive after ~0.8MB of DMA, not 8MB
                ws = P["wstage"].tile([128, 3 * CPC], F32, tag="ws")
                nc.sync.dma_start(
                    ws[:].bitcast(F32R),
                    wqkvT[et * 128:(et + 1) * 128, :].bitcast(F32R),
                )
                nc.vector.tensor_copy(
                    w_bf[:, et * 3 * CPC:(et + 1) * 3 * CPC], ws[:]
                )
            xs = P["xstage"].tile([128, 512], F32, tag="xs")
            nc.sync.dma_start(
                xs[:].bitcast(F32R),
                xT[et * 128:(et + 1) * 128,
                   tci * 512:(tci + 1) * 512].bitcast(F32R),
            )
            xb = P["xbf"].tile([128, 512], BF16, tag="xb")
            if et % 2 == 0:
                nc.vector.tensor_copy(xb[:], xs[:])
            else:
                nc.scalar.copy(xb[:], xs[:])
            xbfs.append(xb)
        if tci == 0:
            nc.sync.dma_start(cos_sb[:], cosd[:, :])
            nc.sync.dma_start(sin_sb[:], sind[:, :])
        elif tci == 1 and P.get("post_tci0"):
            P["post_tci0"]()
        # q, k channel-major [hd, 512 tokens] per local head
        for which, dst in ((0, q_sb), (1, k_sb)):
            for h in range(HPC):
                ps = P["ps_qk"].tile([128, 512], F32, tag="psqk")
                for et in range(16):
                    wcol = et * 3 * CPC + which * CPC + h * HD
                    nc.tensor.matmul(
                        ps[:], w_bf[:, wcol:wcol + HD], xbfs[et][:],
                        start=(et == 0), stop=(et == 15),
                    )
                csl = cos_sb[:, sc * 512:(sc + 1) * 512]
                ssl = sin_sb[:, sc * 512:(sc + 1) * 512]
                qc_t = P["s1tmp"].tile([128, 512], F32, tag="ropeqc")
                nc.vector.tensor_tensor(qc_t[:], ps[:], csl, OP.mult)
                rot = P["s1tmp"].tile([128, 512], F32, tag="roperot")
                nc.scalar.copy(rot[0:64, :], ps[64:128, :])
                nc.scalar.copy(rot[64:128, :], ps[0:64, :])
                nc.vector.tensor_tensor(rot[:], rot[:], ssl, OP.mult)
                col = (b * HPC + h) * S + sc * 512
                nc.vector.tensor_tensor(dst[:, col:col + 512], qc_t[:],
                                        rot[:], OP.add)
        # v token-major [t, 256], kept resident in SBUF (no DRAM bounce)
        v_sb = P["v_sb"]
        for ts4 in range(4):
            ps = P["ps_v"].tile([128, CPC], F32, tag="psv")
            for et in range(16):
                wcol = et * 3 * CPC + 2 * CPC
                nc.tensor.matmul(
                    ps[:], xbfs[et][:, ts4 * 128:(ts4 + 1) * 128],
                    w_bf[:, wcol:wcol + CPC],
                    start=(et == 0), stop=(et == 15),
                )
            blk = tci * 4 + ts4
            nc.scalar.copy(v_sb[:, blk * CPC:(blk + 1) * CPC], ps[:])


def _stage4_finalize(nc, P, ones_col, ones_row, g_sb, st_sum, st_sq,
                     gb, rowsum, woT, out):
    s4res = P["s4res"]
    red8 = s4res.tile([128, 2 * NG], F32, tag="red8")
    nc.vector.tensor_reduce(red8[:, 0:NG].bitcast(F32R), st_sum[:], AX.X,
                            OP.add)
    nc.vector.tensor_reduce(
        red8[:, NG:2 * NG].bitcast(F32R),
        st_sq[:].rearrange("p (g c) -> p g c", g=NG), AX.X, OP.add
    )
    stat_ps = P["ps_st"].tile([1, 2 * NG], F32, tag="statp")
    nc.tensor.matmul(stat_ps[:], ones_col[:].bitcast(F32R),
                     red8[:].bitcast(F32R), start=True, stop=True)
    srow = s4res.tile([1, 2 * NG], F32, tag="srow")
    nc.vector.tensor_scalar(out=srow[:], in0=stat_ps[:],
                            scalar1=1.0 / GROUP_N, scalar2=None, op0=OP.mult)
    var_r = s4res.tile([1, NG], F32, tag="var_r")
    m2 = s4res.tile([1, NG], F32, tag="m2")
    nc.vector.tensor_tensor(m2[:], srow[:, 0:NG], srow[:, 0:NG], OP.mult)
    nc.vector.tensor_tensor(var_r[:], srow[:, NG:2 * NG], m2[:], OP.subtract)
    eps_t = s4res.tile([1, 1], F32, tag="eps_t")
    nc.gpsimd.memset(eps_t[:], EPS)
    std_r = s4res.tile([1, NG], F32, tag="std_r")
    nc.scalar.activation(std_r[:], var_r[:], ACTF.Sqrt, bias=eps_t[:])
    ab_row = s4res.tile([1, 2 * NG], F32, tag="ab_row")
    nc.vector.reciprocal(ab_row[:, 0:NG].bitcast(F32R), std_r[:])
    mtmp = s4res.tile([1, NG], F32, tag="mtmp")
    nc.vector.tensor_tensor(mtmp[:], srow[:, 0:NG], ab_row[:, 0:NG], OP.mult)
    nc.vector.tensor_scalar(
        out=ab_row[:, NG:2 * NG].bitcast(F32R), in0=mtmp[:],
        scalar1=-1.0, scalar2=None, op0=OP.mult,
    )  # b_g = -mean*rstd
    ab_ps = P["ps_st"].tile([128, 2 * NG], F32, tag="abp")
    nc.tensor.matmul(ab_ps[:], ones_row[:].bitcast(F32R),
                     ab_row[:].bitcast(F32R), start=True, stop=True)
    ab_bc = s4res.tile([128, 2 * NG], F32, tag="ab_bc")
    nc.scalar.copy(ab_bc[:], ab_ps[:])

    gamma_col = s4res.tile([128, NG], F32, tag="gamma_col")
    nc.sync.dma_start(gamma_col[:],
                      gb[0, :].rearrange("(c p) -> p c", p=128))
    m1_col = s4res.tile([128, NG], F32, tag="m1_col")
    nc.vector.scalar_tensor_tensor(
        out=m1_col[:], in0=gamma_col[:], scalar=(1.0 - INIT_LAMBDA),
        in1=ab_bc[:, 0:NG], op0=OP.mult, op1=OP.mult,
    )
    gamma_row = s4res.tile([1, TPC], F32, tag="gamma_row")
    nc.sync.dma_start(gamma_row[:], gb[0:1, :])
    beta_row = s4res.tile([1, TPC], F32, tag="beta_row")
    nc.sync.dma_start(beta_row[:], gb[1:2, :])
    m2_row = s4res.tile([1, TPC], F32, tag="m2_row")
    m2tmp = s4res.tile([1, TPC], F32, tag="m2tmp")
    nc.vector.tensor_tensor(
        m2tmp[:].rearrange("o (g u) -> o g u", g=NG),
        gamma_row[:].rearrange("o (g u) -> o g u", g=NG),
        ab_row[:, NG:2 * NG].rearrange("o (g u) -> o g u", u=1)
        .to_broadcast([1, NG, HD]),
        OP.mult,
    )
    nc.vector.tensor_tensor(m2tmp[:], m2tmp[:], beta_row[:], OP.add)
    nc.vector.tensor_scalar(out=m2_row[:], in0=m2tmp[:],
                            scalar1=(1.0 - INIT_LAMBDA), scalar2=None,
                            op0=OP.mult)
    m2_bf = s4res.tile([1, TPC], BF16, tag="m2_bf")
    nc.vector.tensor_copy(m2_bf[:], m2_row[:])
    rs_f = s4res.tile([1, E], F32, tag="rs_f")
    nc.sync.dma_start(rs_f[:], rowsum[:, :])
    rs_bf = s4res.tile([1, E], BF16, tag="rs_bf")
    nc.vector.tensor_copy(rs_bf[:], rs_f[:])

    for oc in range(4):
        if oc == 0 and "wots0" in P:
            wots = P["wots0"]
        else:
            wots = []
            for ct in range(16):
                wstg = P["wostage"].tile([128, 512], F32, tag="wstg")
                nc.sync.dma_start(
                    wstg[:].bitcast(F32R),
                    woT[ct * 128:(ct + 1) * 128,
                        oc * 512:(oc + 1) * 512].bitcast(F32R),
                )
                wot = P["wop"].tile([128, 512], BF16, tag="wot")
                nc.vector.tensor_copy(wot[:], wstg[:])
                wots.append(wot)
        for tcg in range(NG):
            # group tcg = (batch tcg//2, block tcg%2); g_sb is b-major
            po = P["ps_p"].tile([128, 512], F32, tag="po")
            for ct in range(16):
                col = ((tcg // 2) * 16 + ct) * (TPC // B) + (tcg % 2) * 128
                nc.tensor.matmul(
                    po[:],
                    g_sb[:, col:col + 128],
                    wots[ct][:],
                    start=(ct == 0), stop=(ct == 15),
                )
            bps = P["ps_b"].tile([128, 512], F32, tag="bps")
            nc.tensor.matmul(
                bps[:], m2_bf[0:1, tcg * 128:(tcg + 1) * 128],
                rs_bf[0:1, oc * 512:(oc + 1) * 512], start=True, stop=True,
            )
            osb = P["s4tmp"].tile([128, 512], F32, tag="osb")
            nc.scalar.activation(osb[:], po[:], ACTF.Copy,
                                 scale=m1_col[:, tcg:tcg + 1])
            nc.vector.tensor_tensor(osb[:], osb[:], bps[:], OP.add)
            nc.sync.dma_start(
                out[tcg * 128:(tcg + 1) * 128, oc * 512:(oc + 1) * 512],
                osb[:],
            )


def build_nc():
    nc = bacc.Bacc("TRN2", target_bir_lowering=False, debug=False,
                   num_devices=NC)

    xT = nc.declare_dram_parameter("xT", [E, T], F32, isOutput=False)
    wqkvT = nc.declare_dram_parameter("wqkvT", [E, 3 * CPC], F32,
                                      isOutput=False)
    woT = nc.declare_dram_parameter("woT", [E, E], F32, isOutput=False)
    cosd = nc.declare_dram_parameter("cosd", [HD, S], F32, isOutput=False)
    sind = nc.declare_dram_parameter("sind", [HD, S], F32, isOutput=False)
    bandm = nc.declare_dram_parameter("bandm", [4, 128, 512], F32,
                                      isOutput=False)
    identm = nc.declare_dram_parameter("identm", [128, 128], F32,
                                       isOutput=False)
    lam_a = nc.declare_dram_parameter("lam_a", [2, HD], F32, isOutput=False)
    lam_b = nc.declare_dram_parameter("lam_b", [2, HD], F32, isOutput=False)
    sgn2 = nc.declare_dram_parameter("sgn2", [2, 1], F32, isOutput=False)
    ones128 = nc.declare_dram_parameter("ones128", [1, 128], F32,
                                        isOutput=False)
    rowsum = nc.declare_dram_parameter("rowsum", [1, E], F32, isOutput=False)
    gb = nc.declare_dram_parameter("gb", [2, TPC], F32, isOutput=False)
    out = nc.declare_dram_parameter("out", [TPC, E], F32, isOutput=True)

    TPB = TPC // B  # 256 tokens per (core, batch)
    a2a_in = [[nc.dram_tensor(f"a2a_in{h}_{b}", [NC, HD, TPB], BF16)
               for b in range(B)] for h in range(HPC)]
    a2a_out = [[nc.dram_tensor(f"a2a_out{h}_{b}", [NC, HD, TPB], BF16)
                for b in range(B)] for h in range(HPC)]

    with tile.TileContext(nc) as tc, \
         nc.allow_low_precision(reason="bf16 matmul paths; fp32 accumulate"), \
         ExitStack() as top:
        small = top.enter_context(tc.tile_pool(name="small", bufs=1))
        ident_bf = small.tile([128, 128], BF16, tag="ident_bf")
        masks_bf = small.tile([128, 4 * 512], BF16, tag="masks_bf")

        qkres = top.enter_context(tc.tile_pool(name="qkres", bufs=1))
        q_sb = qkres.tile([128, 2 * HPC * S], BF16, tag="q_sb")
        k_sb = qkres.tile([128, 2 * HPC * S], BF16, tag="k_sb")
        # V kept resident in SBUF: 32 token tiles x 256 channels, bf16
        v_sb = qkres.tile([128, 32 * CPC], BF16, tag="v_sb")

        # ======== stage 1: QKV + RoPE ========
        with ExitStack() as s1:
            P1 = {}
            P1["s1res"] = s1.enter_context(tc.tile_pool(name="s1res", bufs=1))
            P1["xstage"] = s1.enter_context(tc.tile_pool(name="xstage",
                                                         bufs=5))
            P1["xbf"] = s1.enter_context(tc.tile_pool(name="xbf", bufs=20))
            P1["s1tmp"] = s1.enter_context(tc.tile_pool(name="s1tmp", bufs=6))
            P1["ps_qk"] = s1.enter_context(
                tc.tile_pool(name="ps_qk", bufs=5, space="PSUM"))
            P1["ps_v"] = s1.enter_context(
                tc.tile_pool(name="ps_v", bufs=3, space="PSUM"))

            w_bf = P1["s1res"].tile([128, 16 * 3 * CPC], BF16, tag="w_bf")
            cos_sb = P1["s1res"].tile([HD, S], F32, tag="cos_sb")
            sin_sb = P1["s1res"].tile([HD, S], F32, tag="sin_sb")
            P1["wstage"] = s1.enter_context(tc.tile_pool(name="wstage",
                                                         bufs=3))

            def post_tci0():
                # stage-2 constants staged after the first token chunk so
                # their DMAs don't delay the stage-1 critical path
                idf = P1["wstage"].tile([128, 128], F32, tag="idf",
                                        name="idf")
                nc.sync.dma_start(idf[:].bitcast(F32R),
                                  identm[:, :].bitcast(F32R))
                nc.vector.tensor_copy(ident_bf[:], idf[:])
                mf = P1["wstage"].tile([128, 4 * 512], F32, tag="mf",
                                       name="mf")
                for o in range(4):
                    nc.sync.dma_start(mf[:, o * 512:(o + 1) * 512],
                                      bandm[o, :, :])
                nc.vector.tensor_copy(masks_bf[:], mf[:])

            P1["post_tci0"] = post_tci0
            P1["v_sb"] = v_sb
            _stage1(nc, P1, q_sb, k_sb, w_bf, cos_sb, sin_sb, xT, wqkvT,
                    cosd, sind)

        # ---- constants needed only from stage 2 on: emitted after stage 1
        # so their DMAs and the lambda matmul don't head-of-line block the
        # stage-1 DMA and PE queues ----
        ones_col = small.tile([128, 1], F32, tag="ones_col")
        nc.sync.dma_start(
            ones_col[:].bitcast(F32R),
            ones128[0, :].rearrange("(p o) -> p o", o=1).bitcast(F32R),
        )
        ones_row = small.tile([1, 128], F32, tag="ones_row")
        nc.sync.dma_start(ones_row[:].bitcast(F32R),
                          ones128[:, :].bitcast(F32R))
        ones_col_bf = small.tile([128, 1], BF16, tag="ones_col_bf")
        nc.vector.tensor_copy(ones_col_bf[:], ones_col[:])

        # ---- lambda scalar ----
        la = small.tile([2, HD], F32, tag="la")
        nc.sync.dma_start(la[:], lam_a[:, :])
        lb = small.tile([2, HD], F32, tag="lb")
        nc.sync.dma_start(lb[:], lam_b[:, :])
        prod = small.tile([2, HD], F32, tag="lprod")
        nc.vector.tensor_tensor(prod[:], la[:], lb[:], OP.mult)
        dots = small.tile([2, 1], F32, tag="ldots")
        nc.vector.tensor_reduce(
            dots[:], prod[:].rearrange("p (n u) -> p n u", u=HD), AX.X, OP.add
        )
        lexp = small.tile([2, 1], F32, tag="lexp")
        nc.scalar.activation(lexp[:], dots[:], ACTF.Exp)
        sv = small.tile([2, 1], F32, tag="sv")
        nc.sync.dma_start(sv[:], sgn2[:, :])
        with tc.tile_pool(name="ps_lam", bufs=1, space="PSUM") as ps_lam:
            lam_ps = ps_lam.tile([1, 1], F32, tag="lam_ps")
            nc.tensor.matmul(lam_ps[:], sv[:], lexp[:], start=True, stop=True)
            lam_t = small.tile([1, 1], F32, tag="lam_t")
            nc.vector.tensor_scalar(out=lam_t[:], in0=lam_ps[:],
                                    scalar1=INIT_LAMBDA, scalar2=None,
                                    op0=OP.add)
        lam_col = small.tile([128, 1], F32, tag="lam_col")
        nc.gpsimd.partition_broadcast(lam_col[:], lam_t[0:1, :])
        lamneg_col = small.tile([128, 1], F32, tag="lamneg_col")
        nc.vector.tensor_scalar(out=lamneg_col[:], in0=lam_col[:],
                                scalar1=-1.0, scalar2=None, op0=OP.mult)

        # ======== stage 2 (+ stage 4 SBUF) ========
        with ExitStack() as s2:
            P = {}
            for name, bufs in (("pt0", 5), ("pt1", 5),
                               ("recipp", 3), ("bcsp", 4), ("thp", 4),
                               ("asbp", 3), ("s4res", 1), ("sqscratch", 2),
                               ("wostage", 4), ("wop", 20), ("s4tmp", 4)):
                P[name] = s2.enter_context(tc.tile_pool(name=name, bufs=bufs))
            with ExitStack() as s2p:
                P["ps_sc"] = s2p.enter_context(
                    tc.tile_pool(name="ps_sc", bufs=2, space="PSUM"))
                P["ps_sum"] = s2p.enter_context(
                    tc.tile_pool(name="ps_sum", bufs=1, space="PSUM"))
                P["ps_o"] = s2p.enter_context(
                    tc.tile_pool(name="ps_o", bufs=1, space="PSUM"))

                TPB = TPC // B  # 256
                NGB = TPB // HD  # 2 groups per (core, batch)
                g_sb = P["s4res"].tile([128, B * 16 * TPB], BF16, tag="g_sb")
                st_sum = P["s4res"].tile([128, B * NGB, 16], F32, tag="st_sum")
                st_sq = P["s4res"].tile([128, B * NGB * 16], F32, tag="st_sq")
                def emit_stats(b, parity=None):
                    # G loads + stat partials for batch b (no PSUM).
                    # parity selects channels of one local head (ct%2) whose
                    # A2A has already fired, so they can be emitted early.
                    for ct in range(16):
                        if parity is not None and ct % 2 != parity:
                            continue
                        col = (b * 16 + ct) * TPB
                        nc.sync.dma_start(
                            g_sb[:, col:col + TPB],
                            a2a_out[ct % 2][b][ct // 2, :, :],
                        )
                        gt = g_sb[:, col:col + TPB]
                        nc.vector.tensor_reduce(
                            st_sum[:, b * NGB:(b + 1) * NGB, ct],
                            gt.rearrange("p (g n) -> p g n", g=NGB),
                            AX.X, OP.add,
                        )
                        for g in range(NGB):
                            scr = P["sqscratch"].tile([128, HD], F32,
                                                      tag="sqs", name="sqs")
                            gi = b * NGB + g
                            nc.scalar.activation(
                                scr[:], gt[:, g * HD:(g + 1) * HD],
                                ACTF.Square,
                                accum_out=st_sq[:, gi * 16 + ct:
                                                gi * 16 + ct + 1],
                            )

                for b, h in ((0, 0), (0, 1), (1, 0), (1, 1)):
                    bh = b * HPC + h

                    def v_bh(kt, b=b, h=h):
                        base = (b * 16 + kt) * CPC + h * HD
                        return v_sb[:, base:base + HD]

                    for qc in range(4):
                        _attn_qc(nc, P, q_sb, k_sb, v_bh, bh, qc,
                                 ident_bf, masks_bf, ones_col_bf,
                                 lamneg_col, a2a_in[h][b])
                        if (b, h) == (1, 1) and qc == 0:
                            # b1's h0 channels arrived with A2A(h0,b1);
                            # emit their stats here so only the h1 half
                            # remains after the final collective
                            emit_stats(1, parity=0)
                    nc.gpsimd.collective_compute(
                        "AllToAll",
                        OP.bypass,
                        replica_groups=[list(range(NC))],
                        ins=[a2a_in[h][b].ap().opt()],
                        outs=[a2a_out[h][b].ap().opt()],
                    )
                    if (b, h) == (1, 0):
                        # b0 stats emitted here: their ACT/DVE ops queue
                        # behind (1,0)'s exps, with the b0 A2As long done —
                        # no head-of-line block on the ACT queue
                        emit_stats(0)
                        # prefetch the first projection weight block so the
                        # tail doesn't start with 2MB of wo DMA + converts
                        wots0 = []
                        for ct in range(16):
                            wstg = P["wostage"].tile([128, 512], F32,
                                                     tag="wstg", name="wstg")
                            nc.sync.dma_start(
                                wstg[:].bitcast(F32R),
                                woT[ct * 128:(ct + 1) * 128,
                                    0:512].bitcast(F32R),
                            )
                            wot = P["wop"].tile([128, 512], BF16,
                                                tag="wot", name="wot")
                            nc.vector.tensor_copy(wot[:], wstg[:])
                            wots0.append(wot)
                        P["wots0"] = wots0
                emit_stats(1, parity=1)

            # ======== stage 4: stats finalize + projection ========
            with ExitStack() as s4p:
                P["ps_st"] = s4p.enter_context(
                    tc.tile_pool(name="ps_st", bufs=1, space="PSUM"))
                P["ps_b"] = s4p.enter_context(
                    tc.tile_pool(name="ps_b", bufs=2, space="PSUM"))
                P["ps_p"] = s4p.enter_context(
                    tc.tile_pool(name="ps_p", bufs=4, space="PSUM"))
                _stage4_finalize(nc, P, ones_col, ones_row, g_sb, st_sum,
                                 st_sq, gb, rowsum, woT, out)

    nc.compile()
    return nc


_NC_CACHE = None


def _get_nc():
    global _NC_CACHE
    if _NC_CACHE is None:
        _NC_CACHE = build_nc()
    return _NC_CACHE


def _host_prep(x, w_qkv, wo, lambda_q1, lambda_q2, lambda_k1, lambda_k2,
               gamma, beta):
    x = np.asarray(x, dtype=np.float32)
    w_qkv = np.asarray(w_qkv, dtype=np.float32)
    wo = np.asarray(wo, dtype=np.float32)
    gamma = np.asarray(gamma, dtype=np.float32)
    beta = np.asarray(beta, dtype=np.float32)

    xT = np.ascontiguousarray(x.reshape(T, E).T)
    woT = np.ascontiguousarray(wo.T)
    rowsum = np.ascontiguousarray(wo.sum(axis=1)[None, :].astype(np.float32))

    # RoPE tables, channel-major with sign folded into sin
    inv = 1.0 / (10000.0 ** (np.arange(0, HD, 2, dtype=np.float32) / HD))
    ang = np.arange(S, dtype=np.float32)[:, None] * inv[None, :]  # (S, 64)
    ang = np.concatenate([ang, ang], axis=-1)                     # (S, 128)
    cosd = np.ascontiguousarray(np.cos(ang).T.astype(np.float32))  # (128, S)
    sin_t = np.sin(ang).T.astype(np.float32)
    sind = np.ascontiguousarray(
        np.concatenate([-sin_t[:64], sin_t[64:]], axis=0)
    )

    # band masks: mask_o[ki, qi] = 1.0 iff (o*128 + ki) > qi
    o_idx = np.arange(4)[:, None, None] * 128
    ki = np.arange(128)[None, :, None]
    qi = np.arange(512)[None, None, :]
    bandm = ((o_idx + ki) > qi).astype(np.float32)
    identm = np.eye(128, dtype=np.float32)

    lam_a = np.ascontiguousarray(
        np.stack([lambda_q1, lambda_q2]).astype(np.float32)
    )
    lam_b = np.ascontiguousarray(
        np.stack([lambda_k1, lambda_k2]).astype(np.float32)
    )

    in_maps = []
    for j in range(NC):
        h0 = HPC * j
        rows_q = w_qkv[h0 * HD:(h0 + HPC) * HD, :]
        # fold the 1/sqrt(hd) attention scale into the K projection rows
        rows_k = w_qkv[E + h0 * HD:E + (h0 + HPC) * HD, :] * SCALER
        rows_v = w_qkv[2 * E + h0 * HD:2 * E + (h0 + HPC) * HD, :]
        wqkvT_j = np.ascontiguousarray(
            np.concatenate([rows_q.T, rows_k.T, rows_v.T], axis=1)
            .astype(np.float32)
        )
        # core j owns tokens [j*256, (j+1)*256) of each batch; gamma/beta
        # slices duplicated so group gi=(b,g) indexing stays NG=4 on device
        sl = j * (TPC // B)
        gsl = gamma[sl:sl + TPC // B]
        bsl = beta[sl:sl + TPC // B]
        gbj = np.ascontiguousarray(
            np.stack([np.concatenate([gsl, gsl]), np.concatenate([bsl, bsl])])
        )
        in_maps.append({
            "sgn2": np.array([[1.0], [-1.0]], dtype=np.float32),
            "ones128": np.ones((1, 128), dtype=np.float32),
            "xT": xT,
            "wqkvT": wqkvT_j,
            "woT": woT,
            "rowsum": rowsum,
            "cosd": cosd,
            "sind": sind,
            "bandm": bandm,
            "identm": identm,
            "lam_a": lam_a,
            "lam_b": lam_b,
            "gb": gbj,
        })
    return in_maps


def kernel(x, w_qkv, wo, lambda_q1, lambda_q2, lambda_k1, lambda_k2,
           gamma, beta, _trace=False):
    nc = _get_nc()
    in_maps = _host_prep(x, w_qkv, wo, lambda_q1, lambda_q2, lambda_k1,
                         lambda_k2, gamma, beta)
    res = run_bass_kernel_spmd(nc, in_maps, list(range(NC)), trace=_trace)
    out = np.empty((B, S, E), dtype=np.float32)
    tpb = TPC // B
    for j in range(NC):
        rows = res.results[j]["out"].reshape(B, tpb, E)
        for b in range(B):
            out[b, j * tpb:(j + 1) * tpb, :] = rows[b]
    if _trace:
        kernel.last_results = res
    return out

